# revision 14
# baseline (speedup 1.0000x reference)
"""Trainium2 Bass kernel for nn_Conv2DLayer_16011638080159.

Math: out = C * (x @ weight.sum(0))   with x [524288, 512], weight [9, 512].
Equivalent to a row-wise dot product of x with w_eff = C * weight.sum(0).

Strategy (pure data parallel, per sharding hint):
  - Shard x along the batch axis across 8 NeuronCores (65536 rows each).
  - Host-side prep: fold the tiny K=9 weight sum and the C scale into a
    single [C] vector, replicated to a [128, 8*C] SBUF-ready constant.
  - Per core: stream x in [128 partitions, 16 rows x 512] tiles from HBM
    (4-deep buffering, alternating both HWDGE rings). The kernel is HBM
    bound (~414 us/core measured pure-DMA floor at 8 cores), so compute is
    split to keep every engine below that:
      * Vector engine multiplies x by the replicated weight (fp32
        tensor_tensor, 1x) -- 2-row groups into PSUM for the 14 rows the
        Scalar engine reduces, plus one SBUF group + one segmented
        tensor_reduce for the 2 rows Vector reduces itself.
      * Scalar engine sums each 512-elem row via ACTIVATE(Copy, accum_out)
        reading PSUM (1 elem/cycle @ 1.2 GHz + fixed overhead per row).
  - Row mapping: shard row (p*512 + t*R + r) sits at partition p, tile t,
    slot r, so the per-core result tile [128, 512] is exactly the row-major
    view of the per-core output [65536]; one contiguous DMA writes it out.
"""

import numpy as np

import concourse.bacc as bacc
import concourse.bass as bass
import concourse.tile as tile
from concourse import mybir
from concourse.bass_utils import run_bass_kernel_spmd

B = 524288        # total rows
C = 512           # row length
N_CORES = 8
BS = B // N_CORES  # 65536 rows per core
P = 128            # SBUF partitions
RPP = BS // P      # 512 rows per partition
R = 16             # rows per partition per tile
F = R * C          # 8192 free elems per tile
NT = RPP // R      # 32 tiles per core
K_DVE = 2          # rows per tile reduced on DVE via segmented tensor_reduce
G = 2              # rows per PSUM product group
WREP = 8           # weight replication factor along free dim

_NC_CACHE = None
LAST_RESULT = None  # BassKernelResults of the most recent run (for profiling)


def _build() -> bass.Bass:
    # Bacc (not raw Bass): its compile() pass splits multi-sem waits into
    # EventSemaphore instructions -- the TRN2 ISA allows only 1 wait/inst.
    nc = bacc.Bacc(None, target_bir_lowering=False, debug=False)
    x = nc.dram_tensor("x", [BS, C], mybir.dt.float32, kind="ExternalInput")
    w = nc.dram_tensor("w", [P, WREP * C], mybir.dt.float32, kind="ExternalInput")
    out = nc.dram_tensor("out", [BS], mybir.dt.float32, kind="ExternalOutput")

    # shard row (p*RPP + t*R + r) -> partition p, tile t, free slot (r, c)
    xv = x.rearrange("(p t r) c -> t p (r c)", p=P, t=NT, r=R)
    ov = out.rearrange("(p f) -> p f", p=P)

    n_act = R - K_DVE          # rows per tile reduced on the Scalar engine
    n_grp = n_act // G         # PSUM product groups per tile

    with tile.TileContext(nc) as tc:
        with (
            tc.tile_pool(name="const", bufs=1) as cpool,
            tc.tile_pool(name="xs", bufs=4) as xs,
            tc.tile_pool(name="yp", bufs=4, space="PSUM") as yp,
            tc.tile_pool(name="ysb", bufs=2) as ysb,
            tc.tile_pool(name="scr", bufs=2) as scr,
            tc.tile_pool(name="res", bufs=1) as res,
        ):
            w_t = cpool.tile([P, WREP * C], mybir.dt.float32)
            nc.sync.dma_start(out=w_t[:], in_=w[:, :])
            o_t = res.tile([P, RPP], mybir.dt.float32)
            for t in range(NT):
                x_t = xs.tile([P, F], mybir.dt.float32)
                eng = nc.scalar if t % 2 == 1 else nc.sync
                eng.dma_start(out=x_t[:], in_=xv[t])

                # rows K_DVE..R-1: DVE mul into PSUM groups, ACT accumulates
                for g in range(n_grp):
                    off = (K_DVE + g * G) * C
                    y_g = yp.tile([P, G * C], mybir.dt.float32, tag="y")
                    nc.vector.tensor_mul(
                        y_g[:], x_t[:, off:off + G * C], w_t[:, 0:G * C]
                    )
                    for r in range(G):
                        s_t = scr.tile([P, C], mybir.dt.float32, tag="act_s")
                        col = t * R + K_DVE + g * G + r
                        nc.scalar.activation(
                            out=s_t[:],
                            in_=y_g[:, r * C:(r + 1) * C],
                            func=mybir.ActivationFunctionType.Copy,
                            accum_out=o_t[:, col: col + 1],
                        )

                # rows 0..K_DVE-1: DVE mul + one segmented reduce
                y_2 = ysb.tile([P, K_DVE * C], mybir.dt.float32)
                nc.vector.tensor_mul(
                    y_2[:], x_t[:, 0:K_DVE * C], w_t[:, 0:K_DVE * C]
                )
                nc.vector.tensor_reduce(
                    out=o_t[:, t * R: t * R + K_DVE],
                    in_=y_2[:].rearrange("p (r c) -> p r c", c=C),
                    axis=mybir.AxisListType.X,
                    op=mybir.AluOpType.add,
                )
            nc.sync.dma_start(out=ov, in_=o_t[:])
    nc.finalize()
    return nc


def kernel(x: np.ndarray, weight: np.ndarray) -> np.ndarray:
    global _NC_CACHE, LAST_RESULT
    x = np.ascontiguousarray(np.asarray(x), dtype=np.float32)
    weight = np.asarray(weight, dtype=np.float32)

    w_eff = (C * weight.sum(axis=0)).astype(np.float32)      # [C]
    w_rep = np.ascontiguousarray(np.tile(w_eff, (P, WREP)))  # [P, WREP*C]

    if _NC_CACHE is None:
        _NC_CACHE = _build()

    in_maps = [
        {"x": x[i * BS:(i + 1) * BS], "w": w_rep} for i in range(N_CORES)
    ]
    LAST_RESULT = run_bass_kernel_spmd(
        _NC_CACHE, in_maps, core_ids=list(range(N_CORES))
    )
    return np.concatenate([r["out"] for r in LAST_RESULT.results])


# revision 15
# speedup vs baseline: 1.0574x; 1.0574x over previous
"""Trainium2 Bass kernel for nn_Conv2DLayer_16011638080159.

Math: out = C * (x @ weight.sum(0))   with x [524288, 512], weight [9, 512].
Equivalent to a row-wise dot product of x with w_eff = C * weight.sum(0).

Strategy (pure data parallel, per sharding hint):
  - Shard x along the batch axis across 8 NeuronCores (65536 rows each).
  - Host-side prep: fold the tiny K=9 weight sum and the C scale into a
    single [C] vector, replicated to a [128, 8*C] SBUF-ready constant.
  - Per core: stream x in [128 partitions, 8 rows x 512] tiles from HBM
    with 6-deep buffering, alternating the two HWDGE rings. The kernel is
    HBM bound (~415 us/core pure-DMA floor measured at 8 cores), so the
    row-dot-products are split so each compute engine stays below that:
      * Vector engine: fp32 tensor_tensor multiply of the whole tile by
        the replicated weight (1x mode), plus a segmented tensor_reduce
        for 1 of the 8 rows  (~320 us/core busy).
      * Scalar engine: the other 7 rows via ACTIVATE(Copy, accum_out),
        which sums 512 elems/row at 1 elem/cycle (~355 us/core busy).
  - Row mapping: shard row (p*512 + t*R + r) sits at partition p, tile t,
    slot r, so the per-core result tile [128, 512] is exactly the row-major
    view of the per-core output [65536]; one contiguous DMA writes it out.
"""

import numpy as np

import concourse.bacc as bacc
import concourse.bass as bass
import concourse.tile as tile
from concourse import mybir
from concourse.bass_utils import run_bass_kernel_spmd

B = 524288        # total rows
C = 512           # row length
N_CORES = 8
BS = B // N_CORES  # 65536 rows per core
P = 128            # SBUF partitions
RPP = BS // P      # 512 rows per partition
R = 8              # rows per partition per tile
F = R * C          # 4096 free elems per tile
NT = RPP // R      # 64 tiles per core
K_DVE = 1          # rows per tile reduced on DVE via segmented tensor_reduce

_NC_CACHE = None
LAST_RESULT = None  # BassKernelResults of the most recent run (for profiling)


def _build() -> bass.Bass:
    # Bacc (not raw Bass): its compile() pass splits multi-sem waits into
    # EventSemaphore instructions -- the TRN2 ISA allows only 1 wait/inst.
    nc = bacc.Bacc(None, target_bir_lowering=False, debug=False)
    x = nc.dram_tensor("x", [BS, C], mybir.dt.float32, kind="ExternalInput")
    w = nc.dram_tensor("w", [P, F], mybir.dt.float32, kind="ExternalInput")
    out = nc.dram_tensor("out", [BS], mybir.dt.float32, kind="ExternalOutput")

    # shard row (p*RPP + t*R + r) -> partition p, tile t, free slot (r, c)
    xv = x.rearrange("(p t r) c -> t p (r c)", p=P, t=NT, r=R)
    ov = out.rearrange("(p f) -> p f", p=P)

    n_act = R - K_DVE  # rows per tile reduced on the Scalar engine

    with tile.TileContext(nc) as tc:
        with (
            tc.tile_pool(name="const", bufs=1) as cpool,
            tc.tile_pool(name="xs", bufs=6) as xs,
            tc.tile_pool(name="ys", bufs=3) as ys,
            tc.tile_pool(name="scr", bufs=2) as scr,
            tc.tile_pool(name="res", bufs=1) as res,
        ):
            w_t = cpool.tile([P, F], mybir.dt.float32)
            nc.sync.dma_start(out=w_t[:], in_=w[:, :])
            o_t = res.tile([P, RPP], mybir.dt.float32)
            for t in range(NT):
                x_t = xs.tile([P, F], mybir.dt.float32)
                eng = nc.scalar if t % 2 == 1 else nc.sync
                eng.dma_start(out=x_t[:], in_=xv[t])

                # one fp32 TT multiply for the whole tile
                y_t = ys.tile([P, F], mybir.dt.float32)
                nc.vector.tensor_mul(y_t[:], x_t[:], w_t[:])

                # ACT accumulates rows K_DVE..R-1 (one 512-sum per row)
                for r in range(n_act):
                    s_t = scr.tile([P, C], mybir.dt.float32, tag="act_s")
                    col = t * R + K_DVE + r
                    nc.scalar.activation(
                        out=s_t[:],
                        in_=y_t[:, (K_DVE + r) * C:(K_DVE + r + 1) * C],
                        func=mybir.ActivationFunctionType.Copy,
                        accum_out=o_t[:, col: col + 1],
                    )

                # DVE reduces rows 0..K_DVE-1 in one segmented reduce
                nc.vector.tensor_reduce(
                    out=o_t[:, t * R: t * R + K_DVE],
                    in_=y_t[:, 0:K_DVE * C].rearrange("p (r c) -> p r c", c=C),
                    axis=mybir.AxisListType.X,
                    op=mybir.AluOpType.add,
                )
            nc.sync.dma_start(out=ov, in_=o_t[:])
    nc.finalize()
    return nc


def kernel(x: np.ndarray, weight: np.ndarray) -> np.ndarray:
    global _NC_CACHE, LAST_RESULT
    x = np.ascontiguousarray(np.asarray(x), dtype=np.float32)
    weight = np.asarray(weight, dtype=np.float32)

    w_eff = (C * weight.sum(axis=0)).astype(np.float32)   # [C]
    w_rep = np.ascontiguousarray(np.tile(w_eff, (P, R)))  # [P, F]

    if _NC_CACHE is None:
        _NC_CACHE = _build()

    in_maps = [
        {"x": x[i * BS:(i + 1) * BS], "w": w_rep} for i in range(N_CORES)
    ]
    LAST_RESULT = run_bass_kernel_spmd(
        _NC_CACHE, in_maps, core_ids=list(range(N_CORES))
    )
    return np.concatenate([r["out"] for r in LAST_RESULT.results])


# revision 16
# speedup vs baseline: 1.4670x; 1.3873x over previous
"""Trainium2 Bass kernel for nn_Conv2DLayer_16011638080159.

Math: out = C * (x @ weight.sum(0))   with x [524288, 512], weight [9, 512].
Equivalent to a row-wise dot product of x with w_eff = C * weight.sum(0).

Strategy (pure data parallel, per sharding hint):
  - Shard x along the batch axis across 8 NeuronCores (65536 rows each).
  - Host-side prep: fold the tiny K=9 weight sum and the C scale into a
    single [C] vector, replicated to a [128, 8*C] SBUF-ready constant.
  - Per core: stream x in [128 partitions, 8 rows x 512] tiles from HBM
    with 6-deep buffering, alternating the two HWDGE rings. The kernel is
    HBM bound (~415 us/core pure-DMA floor measured at 8 cores), so the
    row-dot-products are split so each compute engine stays below that:
      * Vector engine: fp32 tensor_tensor multiply of the whole tile by
        the replicated weight (1x mode), plus a segmented tensor_reduce
        for 1 of the 8 rows  (~320 us/core busy).
      * Scalar engine: the other 7 rows via ACTIVATE(Copy, accum_out),
        which sums 512 elems/row at 1 elem/cycle (~355 us/core busy).
  - Row mapping: shard row (p*512 + t*R + r) sits at partition p, tile t,
    slot r, so the per-core result tile [128, 512] is exactly the row-major
    view of the per-core output [65536]; one contiguous DMA writes it out.
"""

import numpy as np

import concourse.bacc as bacc
import concourse.bass as bass
import concourse.tile as tile
from concourse import mybir
from concourse.bass_utils import run_bass_kernel_spmd

B = 524288        # total rows
C = 512           # row length
N_CORES = 8
BS = B // N_CORES  # 65536 rows per core
P = 128            # SBUF partitions
RPP = BS // P      # 512 rows per partition
R = 8              # rows per partition per tile
F = R * C          # 4096 free elems per tile
NT = RPP // R      # 64 tiles per core
K_DVE = 1          # rows per tile reduced on DVE via segmented tensor_reduce

_NC_CACHE = None
LAST_RESULT = None  # BassKernelResults of the most recent run (for profiling)


def _build() -> bass.Bass:
    # Bacc (not raw Bass): its compile() pass splits multi-sem waits into
    # EventSemaphore instructions -- the TRN2 ISA allows only 1 wait/inst.
    nc = bacc.Bacc(None, target_bir_lowering=False, debug=False)
    x = nc.dram_tensor("x", [BS, C], mybir.dt.float32, kind="ExternalInput")
    w = nc.dram_tensor("w", [P, F], mybir.dt.float32, kind="ExternalInput")
    out = nc.dram_tensor("out", [BS], mybir.dt.float32, kind="ExternalOutput")

    # shard row (p*RPP + t*R + r) -> partition p, tile t, free slot (r, c)
    xv = x.rearrange("(p t r) c -> t p (r c)", p=P, t=NT, r=R)
    ov = out.rearrange("(p f) -> p f", p=P)

    n_act = R - K_DVE  # rows per tile reduced on the Scalar engine

    with tile.TileContext(nc) as tc:
        with (
            tc.tile_pool(name="const", bufs=1) as cpool,
            tc.tile_pool(name="xs", bufs=6) as xs,
            tc.tile_pool(name="ys", bufs=3) as ys,
            tc.tile_pool(name="scr", bufs=2) as scr,
            tc.tile_pool(name="res", bufs=1) as res,
        ):
            w_t = cpool.tile([P, F], mybir.dt.float32)
            nc.sync.dma_start(out=w_t[:], in_=w[:, :])
            o_t = res.tile([P, RPP], mybir.dt.float32)
            for t in range(NT):
                # All x DMAs go on the SP HWDGE ring: SP has no compute, so
                # DMA issue is never queued behind engine work (issuing from
                # nc.scalar stalls the DMA behind pending ACTIVATEs).
                x_t = xs.tile([P, F], mybir.dt.float32)
                nc.sync.dma_start(out=x_t[:], in_=xv[t])

                # one fp32 TT multiply for the whole tile
                y_t = ys.tile([P, F], mybir.dt.float32)
                nc.vector.tensor_mul(y_t[:], x_t[:], w_t[:])

                # ACT accumulates rows K_DVE..R-1 (one 512-sum per row)
                for r in range(n_act):
                    s_t = scr.tile([P, C], mybir.dt.float32, tag="act_s")
                    col = t * R + K_DVE + r
                    nc.scalar.activation(
                        out=s_t[:],
                        in_=y_t[:, (K_DVE + r) * C:(K_DVE + r + 1) * C],
                        func=mybir.ActivationFunctionType.Copy,
                        accum_out=o_t[:, col: col + 1],
                    )

                # DVE reduces rows 0..K_DVE-1 in one segmented reduce
                nc.vector.tensor_reduce(
                    out=o_t[:, t * R: t * R + K_DVE],
                    in_=y_t[:, 0:K_DVE * C].rearrange("p (r c) -> p r c", c=C),
                    axis=mybir.AxisListType.X,
                    op=mybir.AluOpType.add,
                )
            nc.sync.dma_start(out=ov, in_=o_t[:])
    nc.finalize()
    return nc


def kernel(x: np.ndarray, weight: np.ndarray) -> np.ndarray:
    global _NC_CACHE, LAST_RESULT
    x = np.ascontiguousarray(np.asarray(x), dtype=np.float32)
    weight = np.asarray(weight, dtype=np.float32)

    w_eff = (C * weight.sum(axis=0)).astype(np.float32)   # [C]
    w_rep = np.ascontiguousarray(np.tile(w_eff, (P, R)))  # [P, F]

    if _NC_CACHE is None:
        _NC_CACHE = _build()

    in_maps = [
        {"x": x[i * BS:(i + 1) * BS], "w": w_rep} for i in range(N_CORES)
    ]
    LAST_RESULT = run_bass_kernel_spmd(
        _NC_CACHE, in_maps, core_ids=list(range(N_CORES))
    )
    return np.concatenate([r["out"] for r in LAST_RESULT.results])


# revision 17
# speedup vs baseline: 1.5245x; 1.0392x over previous
"""Trainium2 Bass kernel for nn_Conv2DLayer_16011638080159.

Math: out = C * (x @ weight.sum(0))   with x [524288, 512], weight [9, 512].
Equivalent to a row-wise dot product of x with w_eff = C * weight.sum(0).

Strategy (pure data parallel, per sharding hint):
  - Shard x along the batch axis across 8 NeuronCores (65536 rows each).
  - Host-side prep: fold the tiny K=9 weight sum and the C scale into a
    single [C] vector, replicated to a [128, 8*C] SBUF-ready constant.
  - Per core: stream x in [128 partitions, 8 rows x 512] tiles from HBM
    with 6-deep buffering, alternating the two HWDGE rings. The kernel is
    HBM bound (~415 us/core pure-DMA floor measured at 8 cores), so the
    row-dot-products are split so each compute engine stays below that:
      * Vector engine: fp32 tensor_tensor multiply of the whole tile by
        the replicated weight (1x mode), plus a segmented tensor_reduce
        for 1 of the 8 rows  (~320 us/core busy).
      * Scalar engine: the other 7 rows via ACTIVATE(Copy, accum_out),
        which sums 512 elems/row at 1 elem/cycle (~355 us/core busy).
  - Row mapping: shard row (p*512 + t*R + r) sits at partition p, tile t,
    slot r, so the per-core result tile [128, 512] is exactly the row-major
    view of the per-core output [65536]; one contiguous DMA writes it out.
"""

import numpy as np

import concourse.bacc as bacc
import concourse.bass as bass
import concourse.tile as tile
from concourse import mybir
from concourse.bass_utils import run_bass_kernel_spmd

B = 524288        # total rows
C = 512           # row length
N_CORES = 8
BS = B // N_CORES  # 65536 rows per core
P = 128            # SBUF partitions
RPP = BS // P      # 512 rows per partition
R = 8              # rows per partition per tile
F = R * C          # 4096 free elems per tile
NT = RPP // R      # 64 tiles per core
K_DVE = 1          # rows per tile reduced on DVE via segmented tensor_reduce

_NC_CACHE = None
LAST_RESULT = None  # BassKernelResults of the most recent run (for profiling)


def _build() -> bass.Bass:
    # Bacc (not raw Bass): its compile() pass splits multi-sem waits into
    # EventSemaphore instructions -- the TRN2 ISA allows only 1 wait/inst.
    nc = bacc.Bacc(None, target_bir_lowering=False, debug=False)
    x = nc.dram_tensor("x", [BS, C], mybir.dt.float32, kind="ExternalInput")
    w = nc.dram_tensor("w", [P, F], mybir.dt.float32, kind="ExternalInput")
    out = nc.dram_tensor("out", [BS], mybir.dt.float32, kind="ExternalOutput")

    # shard row (p*RPP + t*R + r) -> partition p, tile t, free slot (r, c)
    xv = x.rearrange("(p t r) c -> t p (r c)", p=P, t=NT, r=R)
    ov = out.rearrange("(p f) -> p f", p=P)

    n_act = R - K_DVE  # rows per tile reduced on the Scalar engine

    with tile.TileContext(nc) as tc:
        with (
            tc.tile_pool(name="const", bufs=1) as cpool,
            tc.tile_pool(name="xs", bufs=6) as xs,
            tc.tile_pool(name="ys", bufs=4) as ys,
            tc.tile_pool(name="scr", bufs=2) as scr,
            tc.tile_pool(name="res", bufs=1) as res,
        ):
            w_t = cpool.tile([P, F], mybir.dt.float32)
            nc.sync.dma_start(out=w_t[:], in_=w[:, :])
            o_t = res.tile([P, RPP], mybir.dt.float32)
            for t in range(NT):
                # All x DMAs go on the SP HWDGE ring: SP has no compute, so
                # DMA issue is never queued behind engine work (issuing from
                # nc.scalar stalls the DMA behind pending ACTIVATEs).
                x_t = xs.tile([P, F], mybir.dt.float32)
                nc.sync.dma_start(out=x_t[:], in_=xv[t])

                # one fp32 TT multiply for the whole tile
                y_t = ys.tile([P, F], mybir.dt.float32)
                nc.vector.tensor_mul(y_t[:], x_t[:], w_t[:])

                # ACT accumulates rows K_DVE..R-1 (one 512-sum per row)
                for r in range(n_act):
                    s_t = scr.tile([P, C], mybir.dt.float32, tag="act_s")
                    col = t * R + K_DVE + r
                    nc.scalar.activation(
                        out=s_t[:],
                        in_=y_t[:, (K_DVE + r) * C:(K_DVE + r + 1) * C],
                        func=mybir.ActivationFunctionType.Copy,
                        accum_out=o_t[:, col: col + 1],
                    )

                # DVE reduces rows 0..K_DVE-1 in one segmented reduce
                nc.vector.tensor_reduce(
                    out=o_t[:, t * R: t * R + K_DVE],
                    in_=y_t[:, 0:K_DVE * C].rearrange("p (r c) -> p r c", c=C),
                    axis=mybir.AxisListType.X,
                    op=mybir.AluOpType.add,
                )
            nc.sync.dma_start(out=ov, in_=o_t[:])
    nc.finalize()
    return nc


def kernel(x: np.ndarray, weight: np.ndarray) -> np.ndarray:
    global _NC_CACHE, LAST_RESULT
    x = np.ascontiguousarray(np.asarray(x), dtype=np.float32)
    weight = np.asarray(weight, dtype=np.float32)

    w_eff = (C * weight.sum(axis=0)).astype(np.float32)   # [C]
    w_rep = np.ascontiguousarray(np.tile(w_eff, (P, R)))  # [P, F]

    if _NC_CACHE is None:
        _NC_CACHE = _build()

    in_maps = [
        {"x": x[i * BS:(i + 1) * BS], "w": w_rep} for i in range(N_CORES)
    ]
    LAST_RESULT = run_bass_kernel_spmd(
        _NC_CACHE, in_maps, core_ids=list(range(N_CORES))
    )
    return np.concatenate([r["out"] for r in LAST_RESULT.results])
